# revision 1
# baseline (speedup 1.0000x reference)
"""Trainium2 Bass kernel for nn_DeformLikeASPPConv (8-core data parallel).

Self-contained: kernel(**inputs) takes the full-batch inputs and returns the
full output. One sample per NeuronCore. See emit() for the device pipeline.
"""
import sys
if "/opt/trn_rl_repo" not in sys.path:
    sys.path.insert(0, "/opt/trn_rl_repo")
import numpy as np
import ml_dtypes
import concourse.bass as bass
import concourse.bacc as bacc
import concourse.tile as tile
import concourse.mybir as mybir
from concourse import bass_utils

N_CORES = 8
H, W = 256, 256


NPBF16 = ml_dtypes.bfloat16
C = 64
DIL = 12
BN_EPS = 1e-5


def conv2d_np(x, w, dilation, padding):
    # x: [Cin, H, W], w: [Cout, Cin, 3, 3]
    cin, H, W = x.shape
    cout = w.shape[0]
    p = padding
    xp = np.zeros((cin, H + 2 * p, W + 2 * p), np.float32)
    xp[:, p:p + H, p:p + W] = x
    y = np.zeros((cout, H, W), np.float32)
    for r in range(3):
        for s in range(3):
            sh = xp[:, r * dilation:r * dilation + H,
                    s * dilation:s * dilation + W]
            y += np.einsum('oc,chw->ohw', w[:, :, r, s], sh,
                           optimize=True).astype(np.float32)
    return y


def reference_np(x, offset_w, offset_b, conv_w, bn_gamma, bn_beta, bn_mean,
                 bn_var):
    # x: [C, H, W] single sample
    c, H, W = x.shape
    off = conv2d_np(x, offset_w, 1, 1) + offset_b[:, None, None]
    off = np.tanh(off) * 2.0
    gx = np.linspace(-1.0, 1.0, W, dtype=np.float32)
    gy = np.linspace(-1.0, 1.0, H, dtype=np.float32)
    sx = gx[None, :] + off[0] / max(float(W - 1), 1.0) * 2.0
    sy = gy[:, None] + off[1] / max(float(H - 1), 1.0) * 2.0
    ix = np.clip((sx + 1) * (W - 1) * 0.5, 0, W - 1)
    iy = np.clip((sy + 1) * (H - 1) * 0.5, 0, H - 1)
    x0 = np.floor(ix).astype(np.int32)
    y0 = np.floor(iy).astype(np.int32)
    wx = ix - x0
    wy = iy - y0
    x1 = np.minimum(x0 + 1, W - 1)
    y1 = np.minimum(y0 + 1, H - 1)
    v00 = x[:, y0, x0]
    v01 = x[:, y0, x1]
    v10 = x[:, y1, x0]
    v11 = x[:, y1, x1]
    top = v00 * (1 - wx) + v01 * wx
    bot = v10 * (1 - wx) + v11 * wx
    warped = top * (1 - wy) + bot * wy
    y = conv2d_np(warped.astype(np.float32), conv_w, DIL, DIL)
    inv = bn_gamma / np.sqrt(bn_var + BN_EPS)
    y = y * inv[:, None, None] + (bn_beta - bn_mean * inv)[:, None, None]
    return np.maximum(y, 0)


def prep_core_inputs(x, offset_w, offset_b, conv_w, bn_gamma, bn_beta,
                     bn_mean, bn_var):
    """x: [C, H, W] fp32 one sample -> dict of kernel inputs."""
    c, H, W = x.shape
    N = H * W
    CF = N // 128
    x_cm = x.reshape(C, N).astype(np.float32)
    pm = np.ascontiguousarray(x.reshape(C, N).T).astype(NPBF16)  # [N, C]
    p = np.arange(N)
    x_pm4 = np.concatenate([
        pm[np.minimum(p + d, N - 1)] for d in (0, 1, W, W + 1)],
        axis=1)  # [N, 4C]
    wo18 = np.zeros((C, 18), np.float32)
    for t in range(9):
        r, s = t // 3, t % 3
        for o in range(2):
            wo18[:, 2 * t + o] = offset_w[o, :, r, s]
    sel18 = np.zeros((18, 2), np.float32)
    for t in range(9):
        for o in range(2):
            sel18[2 * t + o, o] = 1.0
    inv = (bn_gamma / np.sqrt(bn_var + BN_EPS)).astype(np.float32)
    wmf = conv_w * inv[:, None, None, None]  # [Cout, Cin, 3, 3]
    wm1 = np.zeros((C, 3 * C), np.float32)
    wm1a = np.zeros((C, 3 * C), np.float32)
    wm1b = np.zeros((C, 3 * C), np.float32)
    wm2 = np.zeros((2 * C, 3 * C), np.float32)
    for gs in range(3):  # ds index 0,1,2 -> shift -1,0,+1
        wm1[:, gs * C:(gs + 1) * C] = wmf[:, :, 1, gs].T
        wm1a[:, gs * C:(gs + 1) * C] = wmf[:, :, 0, gs].T
        wm1b[:, gs * C:(gs + 1) * C] = wmf[:, :, 2, gs].T
        wm2[0:C, gs * C:(gs + 1) * C] = wmf[:, :, 0, gs].T
        wm2[C:2 * C, gs * C:(gs + 1) * C] = wmf[:, :, 2, gs].T
    # order in kern.py: gcol = (ds+1)*C with ds in (0,-1,1) -> gs = ds+1
    biasy = (bn_beta - bn_mean * inv).astype(np.float32).reshape(C, 1)
    pix = np.arange(N).reshape(128, CF)  # compact: partition p -> pixels
    jmap = (pix % W).astype(np.float32)
    imap = (pix // W).astype(np.float32)
    return {
        "x_cm": x_cm,
        "x_pm4": x_pm4,
        "wo18": wo18.astype(np.float32),
        "sel18": sel18.astype(np.float32),
        "wm1": wm1.astype(NPBF16),
        "wm1a": wm1a.astype(NPBF16),
        "wm1b": wm1b.astype(NPBF16),
        "wm2": wm2.astype(NPBF16),
        "offb": offset_b.astype(np.float32).reshape(2, 1),
        "biasy": biasy,
        "jmap": jmap,
        "imap": imap,
    }


IN_SPECS = [
    ("x_cm", (C, None), np.float32),
    ("x_pm4", (None, 4 * C), NPBF16),
    ("wo18", (C, 18), np.float32),
    ("sel18", (18, 2), np.float32),
    ("wm1", (C, 3 * C), NPBF16),
    ("wm1a", (C, 3 * C), NPBF16),
    ("wm1b", (C, 3 * C), NPBF16),
    ("wm2", (2 * C, 3 * C), NPBF16),
    ("offb", (2, 1), np.float32),
    ("biasy", (C, 1), np.float32),
    ("jmap", (128, None), np.float32),
    ("imap", (128, None), np.float32),
]



F32 = mybir.dt.float32
BF16 = mybir.dt.bfloat16
I16 = mybir.dt.int16
I32 = mybir.dt.int32
ALU = mybir.AluOpType
AF = mybir.ActivationFunctionType

C = 64
DIL = 12


def emit(tc, io, H, W):
    nc = tc.nc
    N = H * W
    CF = N // 128
    Po = W + 2
    R_A = 16
    R_B = 8
    M_B = R_B * W
    CLX = (W - 2) + 0.99609375
    CLY = (H - 2) + 0.99609375

    x_cm, x_pm4 = io["x_cm"], io["x_pm4"]
    wo18, sel18 = io["wo18"], io["sel18"]
    wm1, wm1a, wm1b, wm2 = io["wm1"], io["wm1a"], io["wm1b"], io["wm2"]
    offb, biasy = io["offb"], io["biasy"]
    jmap, imap = io["jmap"], io["imap"]
    y_out = io["y"]

    with tc.tile_pool(name="dram", bufs=1, space="DRAM") as dramp, \
         tc.tile_pool(name="consts", bufs=1) as cstp:
        ox_dram = dramp.tile([2, N], F32)
        wxl = dramp.tile([1, N], BF16)
        wyl = dramp.tile([1, N], BF16)
        idxl = dramp.tile([1, N], F32)

        offb_s = cstp.tile([2, 1], F32, tag="offb")
        nc.sync.dma_start(offb_s[:], offb[:])
        biasy_s = cstp.tile([C, 1], F32, tag="biasy")
        nc.sync.dma_start(biasy_s[:], biasy[:])
        wo18_s = cstp.tile([C, 18], F32, tag="wo18")
        nc.sync.dma_start(wo18_s[:], wo18[:])
        sel18_s = cstp.tile([18, 2], F32, tag="sel18")
        nc.sync.dma_start(sel18_s[:], sel18[:])

        # ---------------- Phase A: offset head ----------------
        with tc.tile_pool(name="xa", bufs=2) as xap, \
             tc.tile_pool(name="o18", bufs=2) as o18p, \
             tc.tile_pool(name="al", bufs=2) as alp, \
             tc.tile_pool(name="oxs", bufs=2) as oxsp, \
             tc.tile_pool(name="psA", bufs=3, space="PSUM") as psA, \
             tc.tile_pool(name="psA2", bufs=3, space="PSUM") as psA2:
            for r0 in range(0, H, R_A):
                lo = max(0, r0 - 1)
                hi = min(H, r0 + R_A + 1)
                xt = xap.tile([C, (R_A + 2) * W], F32, tag="xa")
                nc.sync.dma_start(xt[:, 0:(hi - lo) * W],
                                  x_cm[:, lo * W:hi * W])
                o18s = o18p.tile([18, (R_A + 2) * Po], F32, tag="o18")
                o18v = o18s[:].rearrange("p (r w) -> p r w", w=Po)
                nc.vector.memset(o18v[:, :, 0:1], 0.0)
                nc.vector.memset(o18v[:, :, Po - 1:Po], 0.0)
                if r0 == 0:
                    nc.vector.memset(o18v[:, 0:1, :], 0.0)
                if r0 + R_A >= H:
                    nc.vector.memset(o18v[:, R_A + 1:R_A + 2, :], 0.0)
                for cr in range(lo, hi, 2):
                    nrr = min(2, hi - cr)
                    ps = psA.tile([18, 2 * W], F32, tag="psA")
                    nc.tensor.matmul(ps[:, 0:nrr * W], wo18_s[:],
                                     xt[:, (cr - lo) * W:(cr - lo + nrr) * W],
                                     start=True, stop=True)
                    srow = cr - r0 + 1
                    nc.scalar.activation(
                        o18v[:, srow:srow + nrr, 1:W + 1], ps[:, 0:nrr * W],
                        AF.Copy, scale=1.0)
                al = alp.tile([18, R_A * W], F32, tag="al")
                for t in range(9):
                    dr, ds = t // 3 - 1, t % 3 - 1
                    nc.sync.dma_start(
                        al[2 * t:2 * t + 2, :].rearrange(
                            "p (r w) -> p r w", w=W),
                        o18v[2 * t:2 * t + 2, 1 + dr:1 + dr + R_A,
                             1 + ds:1 + ds + W])
                oxs = oxsp.tile([2, R_A * W], F32, tag="oxs")
                for c0 in range(0, R_A * W, 512):
                    ps2 = psA2.tile([2, 512], F32, tag="psA2")
                    nc.tensor.matmul(ps2[:], sel18_s[:], al[:, c0:c0 + 512],
                                     start=True, stop=True)
                    nc.scalar.activation(oxs[:, c0:c0 + 512], ps2[:],
                                         AF.Tanh, bias=offb_s[:], scale=1.0)
                nc.sync.dma_start(ox_dram[:, r0 * W:(r0 + R_A) * W], oxs[:])

        # ---------------- Maps (compact [128, CF]) ----------------
        with tc.tile_pool(name="mp", bufs=1) as mp:
            jm = mp.tile([128, CF], F32, tag="jm")
            nc.sync.dma_start(jm[:], jmap[:])
            im = mp.tile([128, CF], F32, tag="im")
            nc.sync.dma_start(im[:], imap[:])

            def coord_chain(row, base_map, clmax, wl_dram):
                oc = mp.tile([128, CF], F32, tag=f"oc{row}")
                nc.sync.dma_start(
                    oc[:], bass.AP(tensor=ox_dram[:].tensor,
                                   offset=ox_dram[:].offset + row * N,
                                   ap=[[CF, 128], [1, CF]]))
                ic = mp.tile([128, CF], F32, tag=f"ic{row}")
                nc.vector.scalar_tensor_tensor(ic[:], oc[:], 2.0, base_map[:],
                                               ALU.mult, ALU.add)
                nc.vector.tensor_scalar(ic[:], ic[:], 0.0, clmax,
                                        ALU.max, ALU.min)
                i32 = mp.tile([128, CF], I32, tag=f"i32{row}")
                nc.vector.tensor_copy(i32[:], ic[:])
                c0f = mp.tile([128, CF], F32, tag=f"c0f{row}")
                nc.vector.tensor_copy(c0f[:], i32[:])
                wf = mp.tile([128, CF], F32, tag=f"wf{row}")
                nc.vector.tensor_tensor(wf[:], ic[:], c0f[:], ALU.subtract)
                msk = mp.tile([128, CF], F32, tag=f"msk{row}")
                nc.vector.tensor_scalar(msk[:], wf[:], 0.0, None, ALU.is_lt)
                nc.vector.tensor_tensor(c0f[:], c0f[:], msk[:], ALU.subtract)
                nc.vector.tensor_tensor(wf[:], ic[:], c0f[:], ALU.subtract)
                wb = mp.tile([128, CF], BF16, tag=f"wb{row}")
                nc.vector.tensor_copy(wb[:], wf[:])
                nc.sync.dma_start(wl_dram[:], wb[:])
                return c0f

            x0f = coord_chain(0, jm, CLX, wxl)
            y0f = coord_chain(1, im, CLY, wyl)
            idxf = mp.tile([128, CF], F32, tag="idxf")
            nc.vector.scalar_tensor_tensor(idxf[:], y0f[:], float(W), x0f[:],
                                           ALU.mult, ALU.add)
            nc.sync.dma_start(idxl[:], idxf[:])

        # ---------------- Phase B: gather + combine ----------------
        with tc.tile_pool(name="w2", bufs=1) as w2p:
            W2 = w2p.tile([128, N + 2 * W], BF16, tag="W2")
            with tc.tile_pool(name="gb", bufs=2) as gbp, \
                 tc.tile_pool(name="wtb", bufs=2) as wtp, \
                 tc.tile_pool(name="ixb", bufs=2) as ixp, \
                 tc.tile_pool(name="lcb", bufs=2) as lcp:
                for r0 in range(0, H, R_B):
                    base_px = max(0, r0 - 2) * W
                    idxt = ixp.tile([16, M_B // 16], F32, tag="ixf")
                    nc.sync.dma_start(
                        idxt[:], bass.AP(tensor=idxl[:].tensor,
                                         offset=idxl[:].offset + r0 * W,
                                         ap=[[1, 16], [16, M_B // 16]]))
                    sep = ixp.tile([16, M_B // 16], F32, tag="ixs")
                    nc.vector.tensor_scalar(sep[:], idxt[:], float(base_px),
                                            None, ALU.subtract)
                    i16 = ixp.tile([128, M_B // 16], I16, tag="ix16")
                    nc.vector.tensor_copy(i16[0:16, :], sep[:])
                    for rep in range(1, 8):
                        nc.sync.dma_start(i16[16 * rep:16 * rep + 16, :],
                                          i16[0:16, :])
                    g = gbp.tile([128, 2, M_B], BF16, tag="g")
                    nc.gpsimd.dma_gather(
                        g[:], bass.AP(tensor=x_pm4[:].tensor,
                                      offset=x_pm4[:].offset + base_px * 4 * C,
                                      ap=[[4 * C, N - base_px], [1, 4 * C]]),
                        i16[:], M_B, M_B, 4 * C, transpose=True,
                        single_packet=False)
                    wyt = wtp.tile([128, M_B], BF16, tag="wy")
                    nc.sync.dma_start(
                        wyt[:], bass.AP(tensor=wyl[:].tensor,
                                        offset=wyl[:].offset + r0 * W,
                                        ap=[[0, 128], [1, M_B]]))
                    wxt = wtp.tile([64, M_B], BF16, tag="wx")
                    nc.sync.dma_start(
                        wxt[:], bass.AP(tensor=wxl[:].tensor,
                                        offset=wxl[:].offset + r0 * W,
                                        ap=[[0, 64], [1, M_B]]))
                    g0 = g[:, 0, :]
                    g1 = g[:, 1, :]
                    nc.vector.tensor_tensor(g1, g1, g0, ALU.subtract)
                    nc.vector.tensor_tensor(g1, g1, wyt[:], ALU.mult)
                    nc.vector.tensor_tensor(g0, g0, g1, ALU.add)
                    l0 = g[0:64, 0, :]
                    lc = lcp.tile([64, M_B], BF16, tag="lc")
                    nc.scalar.copy(lc[:], g[64:128, 0, :])
                    nc.vector.tensor_tensor(lc[:], lc[:], l0, ALU.subtract)
                    nc.vector.tensor_tensor(lc[:], lc[:], wxt[:], ALU.mult)
                    nc.vector.tensor_tensor(
                        W2[0:64, r0 * W:(r0 + R_B) * W], l0, lc[:], ALU.add)
                    blo = max(2 * DIL, r0)
                    if blo < r0 + R_B:
                        nc.vector.tensor_copy(
                            W2[64:128,
                               (blo - 2 * DIL) * W:(r0 + R_B - 2 * DIL) * W],
                            W2[0:64, blo * W:(r0 + R_B) * W])

            # ---------------- Dilated conv + BN + ReLU ----------------
            with tc.tile_pool(name="wc", bufs=1) as wc, \
                 tc.tile_pool(name="yb", bufs=3) as ybp, \
                 tc.tile_pool(name="psC", bufs=4, space="PSUM") as psC:
                wm1_s = wc.tile([C, 3 * C], BF16, tag="wm1")
                nc.sync.dma_start(wm1_s[:], wm1[:])
                wm1a_s = wc.tile([C, 3 * C], BF16, tag="wm1a")
                nc.sync.dma_start(wm1a_s[:], wm1a[:])
                wm1b_s = wc.tile([C, 3 * C], BF16, tag="wm1b")
                nc.sync.dma_start(wm1b_s[:], wm1b[:])
                wm2_s = wc.tile([128, 3 * C], BF16, tag="wm2")
                nc.sync.dma_start(wm2_s[:], wm2[:])
                yb = None
                for r in range(H):
                    if r % 8 == 0:
                        yb = ybp.tile([C, 8 * W], F32, tag="yb")
                    ps = psC.tile([C, W], F32, tag="psC")
                    seg = {-1: (DIL, W, -DIL), 0: (0, W, 0), 1: (0, W - DIL, DIL)}
                    mms = []
                    for ds in (0, -1, 1):
                        olo, ohi, dsoff = seg[ds]
                        gcol = (ds + 1) * C
                        mms.append((ps[:, olo:ohi], wm1_s[:, gcol:gcol + C],
                                    W2[0:64, r * W + olo + dsoff:
                                       r * W + ohi + dsoff]))
                        if DIL <= r < H - DIL:
                            mms.append(
                                (ps[:, olo:ohi], wm2_s[:, gcol:gcol + C],
                                 W2[:, (r - DIL) * W + olo + dsoff:
                                    (r - DIL) * W + ohi + dsoff]))
                        elif r < DIL:
                            mms.append(
                                (ps[:, olo:ohi], wm1b_s[:, gcol:gcol + C],
                                 W2[0:64, (r + DIL) * W + olo + dsoff:
                                    (r + DIL) * W + ohi + dsoff]))
                        else:
                            mms.append(
                                (ps[:, olo:ohi], wm1a_s[:, gcol:gcol + C],
                                 W2[0:64, (r - DIL) * W + olo + dsoff:
                                    (r - DIL) * W + ohi + dsoff]))
                    for k, (o, l, rr) in enumerate(mms):
                        nc.tensor.matmul(o, l, rr, start=(k == 0),
                                         stop=(k == len(mms) - 1))
                    nc.scalar.activation(yb[:, (r % 8) * W:(r % 8 + 1) * W],
                                         ps[:], AF.Relu, bias=biasy_s[:],
                                         scale=1.0)
                    if r % 8 == 7:
                        nc.sync.dma_start(y_out[:, (r - 7) * W:(r + 1) * W],
                                          yb[:])


_NC_CACHE = {}


def build_nc():
    if "nc" in _NC_CACHE:
        return _NC_CACHE["nc"]
    nc = bacc.Bacc("TRN2", target_bir_lowering=False, debug=False,
                   num_devices=N_CORES)
    N = H * W
    CF = N // 128
    io = {}
    for name, shape, dt in IN_SPECS:
        shape = tuple(s if s is not None else
                      (N if name != "jmap" and name != "imap" else CF)
                      for s in shape)
        mdt = {np.float32: mybir.dt.float32}.get(dt, None)
        if dt is NPBF16:
            mdt = mybir.dt.bfloat16
        elif dt is np.float32:
            mdt = mybir.dt.float32
        io[name] = nc.dram_tensor(name, list(shape), mdt,
                                  kind="ExternalInput").ap()
    io["y"] = nc.dram_tensor("y", [C, N], mybir.dt.float32,
                             kind="ExternalOutput").ap()
    with tile.TileContext(nc) as tc:
        emit(tc, io, H, W)
    nc.compile()
    _NC_CACHE["nc"] = nc
    return nc


def kernel(x, offset_w, offset_b, conv_w, bn_gamma, bn_beta, bn_mean, bn_var):
    x = np.asarray(x, np.float32)
    offset_w = np.asarray(offset_w, np.float32)
    offset_b = np.asarray(offset_b, np.float32)
    conv_w = np.asarray(conv_w, np.float32)
    bn_gamma = np.asarray(bn_gamma, np.float32)
    bn_beta = np.asarray(bn_beta, np.float32)
    bn_mean = np.asarray(bn_mean, np.float32)
    bn_var = np.asarray(bn_var, np.float32)
    B = x.shape[0]
    nc = build_nc()
    base = prep_core_inputs(x[0], offset_w, offset_b, conv_w, bn_gamma,
                            bn_beta, bn_mean, bn_var)
    in_maps = []
    for b in range(B):
        m = dict(base)
        if b > 0:
            xb = x[b]
            N = H * W
            m = dict(base)
            m["x_cm"] = xb.reshape(C, N).astype(np.float32)
            pm = np.ascontiguousarray(
                xb.reshape(C, N).T).astype(NPBF16)
            p = np.arange(N)
            m["x_pm4"] = np.concatenate(
                [pm[np.minimum(p + d, N - 1)] for d in (0, 1, W, W + 1)],
                axis=1)
        in_maps.append(m)
    res = bass_utils.run_bass_kernel_spmd(nc, in_maps,
                                          core_ids=list(range(B)))
    out = np.stack([res.results[b]["y"].reshape(C, H, W) for b in range(B)])
    return out.astype(np.float32)



# revision 18
# speedup vs baseline: 1.8099x; 1.8099x over previous
"""Trainium2 Bass kernel for nn_DeformLikeASPPConv (8-core data parallel).

Self-contained: kernel(**inputs) takes the full-batch inputs and returns the
full output. One sample per NeuronCore. See emit() for the device pipeline.

Pipeline (per core, one sample [64, 256, 256]):
  Phase A: offset head 3x3 conv via the 18-partial trick, all bf16.
  Coords:  tanh + sampling coordinates + compound bilinear weights + i16
           gather indices, computed in compact [128, 512] layout.
  Phase BC: per 16-row block: dma_gather of 4 neighbors -> compound-weight
           bilinear combine (DVE) -> warped rows into a 48-row ring ->
           dilated 3x3 conv (PE) + BN/ReLU -> bf16 output rows.
"""
import sys
if "/opt/trn_rl_repo" not in sys.path:
    sys.path.insert(0, "/opt/trn_rl_repo")
import numpy as np
import ml_dtypes
import concourse.bass as bass
import concourse.bacc as bacc
import concourse.tile as tile
import concourse.mybir as mybir
from concourse import bass_utils

N_CORES = 8
H, W = 256, 256
N = H * W
CF = N // 128  # 512

NPBF16 = ml_dtypes.bfloat16
C = 64
DIL = 12
BN_EPS = 1e-5

RA = 32   # phase A row-block
RB = 16   # phase BC row-block
MB = RB * W  # 4096 pixels per BC block
RING = 48  # warped ring rows


def prep_core_inputs(x, offset_w, offset_b, conv_w, bn_gamma, bn_beta,
                     bn_mean, bn_var):
    """x: [C, H, W] fp32 one sample -> dict of kernel inputs."""
    base = prep_shared(offset_w, offset_b, conv_w, bn_gamma, bn_beta,
                       bn_mean, bn_var)
    base.update(prep_x(x))
    return base


def prep_x(x):
    x_cm = x.reshape(C, N).astype(NPBF16)
    pm = np.ascontiguousarray(x.reshape(C, N).T).astype(NPBF16)  # [N, C]
    p = np.arange(N)
    x_pm4 = np.concatenate([
        pm[np.minimum(p + d, N - 1)] for d in (0, 1, W, W + 1)],
        axis=1)  # [N, 4C]
    return {"x_cm": x_cm, "x_pm4": x_pm4}


def prep_shared(offset_w, offset_b, conv_w, bn_gamma, bn_beta, bn_mean,
                bn_var):
    wo18 = np.zeros((C, 18), np.float32)
    for t in range(9):
        r, s = t // 3, t % 3
        for o in range(2):
            wo18[:, 2 * t + o] = offset_w[o, :, r, s]
    sel18 = np.zeros((18, 2), np.float32)
    for t in range(9):
        for o in range(2):
            sel18[2 * t + o, o] = 1.0
    inv = (bn_gamma / np.sqrt(bn_var + BN_EPS)).astype(np.float32)
    wmf = conv_w * inv[:, None, None, None]  # [Cout, Cin, 3, 3]
    wm1 = np.zeros((C, 3 * C), np.float32)
    wm1a = np.zeros((C, 3 * C), np.float32)
    wm1b = np.zeros((C, 3 * C), np.float32)
    wm2 = np.zeros((2 * C, 3 * C), np.float32)
    for gs in range(3):  # gcol = (ds+1)*C with ds = gs-1
        wm1[:, gs * C:(gs + 1) * C] = wmf[:, :, 1, gs].T
        wm1a[:, gs * C:(gs + 1) * C] = wmf[:, :, 0, gs].T
        wm1b[:, gs * C:(gs + 1) * C] = wmf[:, :, 2, gs].T
        wm2[0:C, gs * C:(gs + 1) * C] = wmf[:, :, 0, gs].T
        wm2[C:2 * C, gs * C:(gs + 1) * C] = wmf[:, :, 2, gs].T
    biasy = (bn_beta - bn_mean * inv).astype(np.float32).reshape(C, 1)
    pix = np.arange(N).reshape(128, CF)
    jmap = (pix % W).astype(np.float32)
    imap = (pix // W).astype(np.float32)
    parts = np.arange(128)
    pbase = np.maximum(0, RB * (parts // 8) - 2).astype(np.float32) * W
    return {
        "wo18": wo18.astype(NPBF16),
        "sel18": sel18.astype(NPBF16),
        "wm1": wm1.astype(NPBF16),
        "wm1a": wm1a.astype(NPBF16),
        "wm1b": wm1b.astype(NPBF16),
        "wm2": wm2.astype(NPBF16),
        "offb2": offset_b.astype(np.float32).reshape(2, 1),
        "biasy": biasy,
        "jmap": jmap,
        "imap": imap,
        "pbase": pbase.reshape(128, 1),
    }


IN_SPECS = [
    ("x_cm", (C, N), NPBF16),
    ("x_pm4", (N, 4 * C), NPBF16),
    ("wo18", (C, 18), NPBF16),
    ("sel18", (18, 2), NPBF16),
    ("wm1", (C, 3 * C), NPBF16),
    ("wm1a", (C, 3 * C), NPBF16),
    ("wm1b", (C, 3 * C), NPBF16),
    ("wm2", (2 * C, 3 * C), NPBF16),
    ("offb2", (2, 1), np.float32),
    ("biasy", (C, 1), np.float32),
    ("jmap", (128, CF), np.float32),
    ("imap", (128, CF), np.float32),
    ("pbase", (128, 1), np.float32),
]

F32 = mybir.dt.float32
BF16 = mybir.dt.bfloat16
I16 = mybir.dt.int16
I32 = mybir.dt.int32
ALU = mybir.AluOpType
AF = mybir.ActivationFunctionType

CLX = (W - 2) + 0.99609375
CLY = (H - 2) + 0.99609375


def emit(tc, io, H_, W_):
    nc = tc.nc
    Po = W + 2

    x_cm, x_pm4 = io["x_cm"], io["x_pm4"]
    wo18, sel18 = io["wo18"], io["sel18"]
    wm1, wm1a, wm1b, wm2 = io["wm1"], io["wm1a"], io["wm1b"], io["wm2"]
    offb2, biasy = io["offb2"], io["biasy"]
    jmap, imap, pbase = io["jmap"], io["imap"], io["pbase"]
    y_out = io["y"]

    with tc.tile_pool(name="dram", bufs=1, space="DRAM") as dramp, \
         tc.tile_pool(name="consts", bufs=1) as cstp:
        # debug builds pass these as ExternalOutputs via io
        ox_dram = io.get("dbg_ox") or dramp.tile([2, N], BF16)
        cmaps = io.get("dbg_cm") or dramp.tile([4, N], BF16)
        idxw = io.get("dbg_ix") or dramp.tile([1, N], I16)

        offb2_s = cstp.tile([2, 1], F32, tag="offb2")
        nc.sync.dma_start(offb2_s[:], offb2[:])
        biasy_s = cstp.tile([C, 1], F32, tag="biasy")
        nc.sync.dma_start(biasy_s[:], biasy[:])
        pbase_s = cstp.tile([128, 1], F32, tag="pbase")
        nc.sync.dma_start(pbase_s[:], pbase[:])
        wo18_s = cstp.tile([C, 18], BF16, tag="wo18")
        nc.sync.dma_start(wo18_s[:], wo18[:])
        sel18_s = cstp.tile([18, 2], BF16, tag="sel18")
        nc.sync.dma_start(sel18_s[:], sel18[:])

        # ---------------- Phase A: offset head (all bf16) ----------------
        with tc.tile_pool(name="xa", bufs=2) as xap, \
             tc.tile_pool(name="o18", bufs=2) as o18p, \
             tc.tile_pool(name="al", bufs=2) as alp, \
             tc.tile_pool(name="oxs", bufs=2) as oxsp, \
             tc.tile_pool(name="psA", bufs=2, space="PSUM") as psA, \
             tc.tile_pool(name="psA2", bufs=2, space="PSUM") as psA2:
            cp_i = 0
            for r0 in range(0, H, RA):
                lo = max(0, r0 - 1)
                hi = min(H, r0 + RA + 1)
                xt = xap.tile([C, (RA + 2) * W], BF16, tag="xa")
                nc.sync.dma_start(xt[:, 0:(hi - lo) * W],
                                  x_cm[:, lo * W:hi * W])
                o18s = o18p.tile([18, (RA + 2) * Po], BF16, tag="o18")
                o18v = o18s[:].rearrange("p (r w) -> p r w", w=Po)
                nc.vector.memset(o18v[:, :, 0:1], 0.0)
                nc.vector.memset(o18v[:, :, Po - 1:Po], 0.0)
                if r0 == 0:
                    nc.vector.memset(o18v[:, 0:1, :], 0.0)
                if r0 + RA >= H:
                    nc.vector.memset(o18v[:, RA + 1:RA + 2, :], 0.0)
                for cr in range(lo, hi, 4):
                    nrr = min(4, hi - cr)
                    ps = psA.tile([18, 4 * W], F32, tag="psA")
                    for j in range(0, nrr, 2):
                        nj = min(2, nrr - j)
                        nc.tensor.matmul(
                            ps[:, j * W:(j + nj) * W], wo18_s[:],
                            xt[:, (cr - lo + j) * W:(cr - lo + j + nj) * W],
                            start=True, stop=True)
                    srow = cr - r0 + 1
                    dst = o18v[:, srow:srow + nrr, 1:W + 1]
                    if cp_i % 2 == 0:
                        nc.vector.tensor_copy(dst, ps[:, 0:nrr * W])
                    else:
                        nc.scalar.activation(dst, ps[:, 0:nrr * W], AF.Copy,
                                             scale=1.0)
                    cp_i += 1
                al = alp.tile([18, RA * W], BF16, tag="al")
                for t in range(9):
                    dr, ds = t // 3 - 1, t % 3 - 1
                    nc.sync.dma_start(
                        al[2 * t:2 * t + 2, :].rearrange(
                            "p (r w) -> p r w", w=W),
                        o18v[2 * t:2 * t + 2, 1 + dr:1 + dr + RA,
                             1 + ds:1 + ds + W])
                oxs = oxsp.tile([2, RA * W], BF16, tag="oxs")
                for c0 in range(0, RA * W, 1024):
                    ps2 = psA2.tile([2, 1024], F32, tag="psA2")
                    for j in range(0, 1024, 512):
                        nc.tensor.matmul(ps2[:, j:j + 512], sel18_s[:],
                                         al[:, c0 + j:c0 + j + 512],
                                         start=True, stop=True)
                    nc.scalar.activation(oxs[:, c0:c0 + 1024], ps2[:],
                                         AF.Tanh, bias=offb2_s[:], scale=1.0)
                nc.sync.dma_start(
                    bass.AP(tensor=ox_dram[:].tensor,
                            offset=ox_dram[:].offset + r0 * W,
                            ap=[[N, 2], [1, RA * W]]),
                    oxs[:])

        # ---------------- Coords (compact [128, CF] layout) --------------
        with tc.tile_pool(name="mp", bufs=1) as mp:
            jm = mp.tile([128, CF], F32, tag="jm")
            nc.sync.dma_start(jm[:], jmap[:])
            im = mp.tile([128, CF], F32, tag="im")
            nc.sync.dma_start(im[:], imap[:])

            def coord_chain(row, base_map, clmax):
                """-> (c0f floor-coord f32, wf frac f32) in compact layout."""
                oc = mp.tile([128, CF], BF16, tag=f"oc{row}")
                nc.sync.dma_start(
                    oc[:], bass.AP(tensor=ox_dram[:].tensor,
                                   offset=ox_dram[:].offset + row * N,
                                   ap=[[CF, 128], [1, CF]]))
                ic = mp.tile([128, CF], F32, tag=f"ic{row}")
                nc.vector.scalar_tensor_tensor(ic[:], oc[:], 2.0, base_map[:],
                                               ALU.mult, ALU.add)
                nc.vector.tensor_scalar(ic[:], ic[:], 0.0, clmax,
                                        ALU.max, ALU.min)
                i32t = mp.tile([128, CF], I32, tag=f"i32{row}")
                nc.vector.tensor_copy(i32t[:], ic[:])
                c0f = mp.tile([128, CF], F32, tag=f"c0f{row}")
                nc.vector.tensor_copy(c0f[:], i32t[:])
                wf = mp.tile([128, CF], F32, tag=f"wf{row}")
                nc.vector.tensor_tensor(wf[:], ic[:], c0f[:], ALU.subtract)
                # hw f32->i32 rounds to nearest; correct to floor
                msk = mp.tile([128, CF], F32, tag=f"msk{row}")
                nc.vector.tensor_scalar(msk[:], wf[:], 0.0, None, ALU.is_lt)
                nc.vector.tensor_tensor(c0f[:], c0f[:], msk[:], ALU.subtract)
                nc.vector.tensor_tensor(wf[:], ic[:], c0f[:], ALU.subtract)
                return c0f, wf

            x0f, wxf = coord_chain(0, jm, CLX)
            y0f, wyf = coord_chain(1, im, CLY)
            for nm, t in (("dbg_x0f", x0f), ("dbg_wxf", wxf),
                          ("dbg_y0f", y0f), ("dbg_wyf", wyf)):
                if io.get(nm) is not None:
                    nc.sync.dma_start(io[nm][:], t[:])

            vx0 = mp.tile([128, CF], F32, tag="vx0")
            nc.vector.tensor_scalar(vx0[:], wxf[:], -1.0, 1.0,
                                    ALU.mult, ALU.add)
            vy0 = mp.tile([128, CF], F32, tag="vy0")
            nc.vector.tensor_scalar(vy0[:], wyf[:], -1.0, 1.0,
                                    ALU.mult, ALU.add)
            cmt = mp.tile([128, 4, CF], BF16, tag="cmt")
            nc.vector.tensor_tensor(cmt[:, 0, :], vy0[:], vx0[:], ALU.mult)
            nc.vector.tensor_tensor(cmt[:, 1, :], vy0[:], wxf[:], ALU.mult)
            nc.vector.tensor_tensor(cmt[:, 2, :], wyf[:], vx0[:], ALU.mult)
            nc.vector.tensor_tensor(cmt[:, 3, :], wyf[:], wxf[:], ALU.mult)
            nc.sync.dma_start(
                bass.AP(tensor=cmaps[:].tensor, offset=cmaps[:].offset,
                        ap=[[CF, 128], [N, 4], [1, CF]]),
                cmt[:])

            idxf = mp.tile([128, CF], F32, tag="idxf")
            nc.vector.scalar_tensor_tensor(idxf[:], y0f[:], float(W), x0f[:],
                                           ALU.mult, ALU.add)
            nc.vector.tensor_scalar(idxf[:], idxf[:], pbase_s[:], None,
                                    ALU.subtract)
            # i16 convert + in-partition (a,b)->(b,a) shuffle so the DRAM
            # write below is stride-1-innermost on both sides.
            # block k (4096 px = partitions 8k..8k+8), local pixel
            # m = q*512 + c, c = 16a+b  ->  dram pos k*4096 + 256*b + 32*q + a
            iiw = mp.tile([128, CF], I16, tag="iiw")
            nc.vector.tensor_copy(
                iiw[:].rearrange("p (b a) -> p b a", a=32),
                idxf[:].rearrange("p (a b) -> p b a", b=16))
            for k in range(N // MB):
                src = iiw[8 * k:8 * k + 8, :].rearrange(
                    "p (b a) -> p b a", a=32)
                nc.sync.dma_start(
                    bass.AP(tensor=idxw[:].tensor,
                            offset=idxw[:].offset + k * MB,
                            ap=[[32, 8], [256, 16], [1, 32]]),
                    src)

        # ---------------- Phase BC: gather + combine + conv --------------
        with tc.tile_pool(name="w2", bufs=1) as w2p, \
             tc.tile_pool(name="wc", bufs=1) as wc:
            W2 = w2p.tile([128, RING * W], BF16, tag="W2")
            wm1_s = wc.tile([C, 3 * C], BF16, tag="wm1")
            nc.sync.dma_start(wm1_s[:], wm1[:])
            wm1a_s = wc.tile([C, 3 * C], BF16, tag="wm1a")
            nc.sync.dma_start(wm1a_s[:], wm1a[:])
            wm1b_s = wc.tile([C, 3 * C], BF16, tag="wm1b")
            nc.sync.dma_start(wm1b_s[:], wm1b[:])
            wm2_s = wc.tile([128, 3 * C], BF16, tag="wm2")
            nc.sync.dma_start(wm2_s[:], wm2[:])

            with tc.tile_pool(name="gb", bufs=2) as gbp, \
                 tc.tile_pool(name="cwb", bufs=2) as cwp, \
                 tc.tile_pool(name="ixb", bufs=2) as ixp, \
                 tc.tile_pool(name="th", bufs=2) as thp, \
                 tc.tile_pool(name="yb", bufs=2) as ybp, \
                 tc.tile_pool(name="psC", bufs=2, space="PSUM") as psC:

                def conv_rows(rlo, rhi):
                    seg = {-1: (DIL, W, -DIL), 0: (0, W, 0),
                           1: (0, W - DIL, DIL)}
                    for r8 in range(rlo, rhi, 8):
                        ps = psC.tile([C, 8 * W], F32, tag="psC")
                        for r in range(r8, r8 + 8):
                            po = (r - r8) * W
                            mms = []
                            for ds in (0, -1, 1):
                                olo, ohi, dsoff = seg[ds]
                                gcol = (ds + 1) * C
                                base = (r % RING) * W
                                mms.append(
                                    (ps[:, po + olo:po + ohi],
                                     wm1_s[:, gcol:gcol + C],
                                     W2[0:64, base + olo + dsoff:
                                        base + ohi + dsoff]))
                                if DIL <= r < H - DIL:
                                    b2 = ((r - DIL) % RING) * W
                                    mms.append(
                                        (ps[:, po + olo:po + ohi],
                                         wm2_s[:, gcol:gcol + C],
                                         W2[:, b2 + olo + dsoff:
                                            b2 + ohi + dsoff]))
                                elif r < DIL:
                                    b2 = ((r + DIL) % RING) * W
                                    mms.append(
                                        (ps[:, po + olo:po + ohi],
                                         wm1b_s[:, gcol:gcol + C],
                                         W2[0:64, b2 + olo + dsoff:
                                            b2 + ohi + dsoff]))
                                else:
                                    b2 = ((r - DIL) % RING) * W
                                    mms.append(
                                        (ps[:, po + olo:po + ohi],
                                         wm1a_s[:, gcol:gcol + C],
                                         W2[0:64, b2 + olo + dsoff:
                                            b2 + ohi + dsoff]))
                            for mi, (o, l, rr) in enumerate(mms):
                                nc.tensor.matmul(o, l, rr, start=(mi == 0),
                                                 stop=(mi == len(mms) - 1))
                        yb = ybp.tile([C, 8 * W], BF16, tag="yb")
                        nc.scalar.activation(yb[:], ps[:], AF.Relu,
                                             bias=biasy_s[:], scale=1.0)
                        nc.sync.dma_start(y_out[:, r8 * W:(r8 + 8) * W],
                                          yb[:])

                for k in range(N // MB):
                    r0 = k * RB
                    base_px = max(0, r0 - 2) * W
                    ixt = ixp.tile([128, MB // 16], I16, tag="ix")
                    nc.sync.dma_start(
                        ixt[:], bass.AP(tensor=idxw[:].tensor,
                                        offset=idxw[:].offset + k * MB,
                                        ap=[[0, 8], [MB // 16, 16],
                                            [1, MB // 16]]))
                    g = gbp.tile([128, 2, MB], BF16, tag="g")
                    nc.gpsimd.dma_gather(
                        g[:], bass.AP(tensor=x_pm4[:].tensor,
                                      offset=x_pm4[:].offset + base_px * 4 * C,
                                      ap=[[4 * C, N - base_px], [1, 4 * C]]),
                        ixt[:], MB, MB, 4 * C, transpose=True,
                        single_packet=False)
                    cw = cwp.tile([128, 2, MB], BF16, tag="cw")
                    for gi in range(2):
                        nc.sync.dma_start(
                            cw[:, gi, :],
                            bass.AP(tensor=cmaps[:].tensor,
                                    offset=cmaps[:].offset + 2 * gi * N
                                    + r0 * W,
                                    ap=[[N, 2], [0, 64], [1, MB]]))
                    g0 = g[:, 0, :]
                    g1 = g[:, 1, :]
                    nc.vector.tensor_tensor(g0, g0, cw[:, 0, :], ALU.mult)
                    nc.vector.tensor_tensor(g1, g1, cw[:, 1, :], ALU.mult)
                    nc.vector.tensor_tensor(g0, g0, g1, ALU.add)
                    th = thp.tile([64, MB], BF16, tag="th")
                    nc.vector.tensor_copy(th[:], g0[64:128])
                    slot = (r0 % RING) * W
                    nc.vector.tensor_tensor(
                        W2[0:64, slot:slot + MB], g0[0:64], th[:],
                        ALU.add)
                    # fill partitions 64:128 (row +24 copies) for slot-rows
                    # [r0-24, r0-8) in two 8-row pieces
                    for s in (r0 - 24, r0 - 16):
                        if s < 0:
                            continue
                        dsl = (s % RING) * W
                        ssl = ((s + 24) % RING) * W
                        nc.vector.tensor_copy(
                            W2[64:128, dsl:dsl + 8 * W],
                            W2[0:64, ssl:ssl + 8 * W])
                    if k >= 1:
                        conv_rows(r0 - RB, r0)
                conv_rows(H - RB, H)


_NC_CACHE = {}


def build_io(nc):
    io = {}
    for name, shape, dt in IN_SPECS:
        mdt = BF16 if dt is NPBF16 else F32
        io[name] = nc.dram_tensor(name, list(shape), mdt,
                                  kind="ExternalInput").ap()
    io["y"] = nc.dram_tensor("y", [C, N], BF16, kind="ExternalOutput").ap()
    return io


def build_nc():
    if "nc" in _NC_CACHE:
        return _NC_CACHE["nc"]
    nc = bacc.Bacc("TRN2", target_bir_lowering=False, debug=False,
                   num_devices=N_CORES)
    io = build_io(nc)
    with tile.TileContext(nc) as tc:
        emit(tc, io, H, W)
    nc.compile()
    _NC_CACHE["nc"] = nc
    return nc


def kernel(x, offset_w, offset_b, conv_w, bn_gamma, bn_beta, bn_mean, bn_var):
    x = np.asarray(x, np.float32)
    offset_w = np.asarray(offset_w, np.float32)
    offset_b = np.asarray(offset_b, np.float32)
    conv_w = np.asarray(conv_w, np.float32)
    bn_gamma = np.asarray(bn_gamma, np.float32)
    bn_beta = np.asarray(bn_beta, np.float32)
    bn_mean = np.asarray(bn_mean, np.float32)
    bn_var = np.asarray(bn_var, np.float32)
    B = x.shape[0]
    nc = build_nc()
    shared = prep_shared(offset_w, offset_b, conv_w, bn_gamma, bn_beta,
                         bn_mean, bn_var)
    in_maps = []
    for b in range(B):
        m = dict(shared)
        m.update(prep_x(x[b]))
        in_maps.append(m)
    res = bass_utils.run_bass_kernel_spmd(nc, in_maps,
                                          core_ids=list(range(B)))
    out = np.stack([
        np.asarray(res.results[b]["y"], dtype=np.float32).reshape(C, H, W)
        for b in range(B)])
    return out


# revision 30
# speedup vs baseline: 2.1718x; 1.1999x over previous
"""Trainium2 Bass kernel for nn_DeformLikeASPPConv (8-core data parallel).

Self-contained: kernel(**inputs) takes the full-batch inputs and returns the
full output. One sample per NeuronCore. See emit() for the device pipeline.

Pipeline (per core, one sample [64, 256, 256]):
  Phase A: offset head 3x3 conv via the 18-partial trick, all bf16.
  Coords:  tanh + sampling coordinates + compound bilinear weights + i16
           gather indices, computed in compact [128, 512] layout.
  Phase BC: per 16-row block: dma_gather of 4 neighbors -> compound-weight
           bilinear combine (DVE) -> warped rows into a 48-row ring ->
           dilated 3x3 conv (PE) + BN/ReLU -> bf16 output rows.
"""
import sys
if "/opt/trn_rl_repo" not in sys.path:
    sys.path.insert(0, "/opt/trn_rl_repo")
import numpy as np
import ml_dtypes
import concourse.bass as bass
import concourse.bacc as bacc
import concourse.tile as tile
import concourse.mybir as mybir
from concourse import bass_utils

N_CORES = 8
H, W = 256, 256
N = H * W
CF = N // 128  # 512

NPBF16 = ml_dtypes.bfloat16
C = 64
DIL = 12
BN_EPS = 1e-5

RA = 64   # phase A row-block
RB = 16   # phase BC row-block
MB = RB * W  # 4096 pixels per BC block
RING = 64  # warped ring rows


def prep_core_inputs(x, offset_w, offset_b, conv_w, bn_gamma, bn_beta,
                     bn_mean, bn_var):
    """x: [C, H, W] fp32 one sample -> dict of kernel inputs."""
    base = prep_shared(offset_w, offset_b, conv_w, bn_gamma, bn_beta,
                       bn_mean, bn_var)
    base.update(prep_x(x))
    return base


def prep_x(x):
    x_cm = x.reshape(C, N).astype(NPBF16)
    pm = np.ascontiguousarray(x.reshape(C, N).T).astype(NPBF16)  # [N, C]
    p = np.arange(N)
    x_pm4 = np.concatenate([
        pm[np.minimum(p + d, N - 1)] for d in (0, 1, W, W + 1)],
        axis=1)  # [N, 4C]
    return {"x_cm": x_cm, "x_pm4": x_pm4}


def prep_shared(offset_w, offset_b, conv_w, bn_gamma, bn_beta, bn_mean,
                bn_var):
    wo18 = np.zeros((C, 32), np.float32)  # cols 18:32 zero-padded so the
    for t in range(9):                    # packed matmuls fill whole
        r, s = t // 3, t % 3              # 32-partition PSUM groups
        for o in range(2):
            wo18[:, 2 * t + o] = offset_w[o, :, r, s]
    sel18 = np.zeros((18, 32), np.float32)
    for t in range(9):
        for o in range(2):
            sel18[2 * t + o, o] = 1.0
    inv = (bn_gamma / np.sqrt(bn_var + BN_EPS)).astype(np.float32)
    wmf = conv_w * inv[:, None, None, None]  # [Cout, Cin, 3, 3]
    wm1 = np.zeros((C, 3 * C), np.float32)
    wm1a = np.zeros((C, 3 * C), np.float32)
    wm1b = np.zeros((C, 3 * C), np.float32)
    wm2 = np.zeros((2 * C, 3 * C), np.float32)
    for gs in range(3):  # gcol = (ds+1)*C with ds = gs-1
        wm1[:, gs * C:(gs + 1) * C] = wmf[:, :, 1, gs].T
        wm1a[:, gs * C:(gs + 1) * C] = wmf[:, :, 0, gs].T
        wm1b[:, gs * C:(gs + 1) * C] = wmf[:, :, 2, gs].T
        wm2[0:C, gs * C:(gs + 1) * C] = wmf[:, :, 0, gs].T
        wm2[C:2 * C, gs * C:(gs + 1) * C] = wmf[:, :, 2, gs].T
    biasy = (bn_beta - bn_mean * inv).astype(np.float32).reshape(C, 1)
    pix = np.arange(N).reshape(128, CF)
    jmap = (pix % W).astype(np.float32)
    imap = (pix // W).astype(np.float32)
    parts = np.arange(128)
    pbase = np.maximum(0, RB * (parts // 8) - 2).astype(np.float32) * W
    return {
        "wo18": wo18.astype(NPBF16),
        "sel18": sel18.astype(NPBF16),
        "wm1": wm1.astype(NPBF16),
        "wm1a": wm1a.astype(NPBF16),
        "wm1b": wm1b.astype(NPBF16),
        "wm2": wm2.astype(NPBF16),
        "offbp": np.broadcast_to(offset_b.astype(np.float32)[None, :],
                                 (128, 2)).copy(),
        "biasy": biasy,
        "jmap": jmap,
        "imap": imap,
        "pbase": pbase.reshape(128, 1),
    }


IN_SPECS = [
    ("x_cm", (C, N), NPBF16),
    ("x_pm4", (N, 4 * C), NPBF16),
    ("wo18", (C, 32), NPBF16),
    ("sel18", (18, 32), NPBF16),
    ("wm1", (C, 3 * C), NPBF16),
    ("wm1a", (C, 3 * C), NPBF16),
    ("wm1b", (C, 3 * C), NPBF16),
    ("wm2", (2 * C, 3 * C), NPBF16),
    ("offbp", (128, 2), np.float32),
    ("biasy", (C, 1), np.float32),
    ("jmap", (128, CF), np.float32),
    ("imap", (128, CF), np.float32),
    ("pbase", (128, 1), np.float32),
]

F32 = mybir.dt.float32
BF16 = mybir.dt.bfloat16
I16 = mybir.dt.int16
I32 = mybir.dt.int32
ALU = mybir.AluOpType
AF = mybir.ActivationFunctionType

CLX = (W - 2) + 0.99609375
CLY = (H - 2) + 0.99609375


def emit(tc, io, H_, W_):
    nc = tc.nc
    Po = W + 2

    x_cm, x_pm4 = io["x_cm"], io["x_pm4"]
    wo18, sel18 = io["wo18"], io["sel18"]
    wm1, wm1a, wm1b, wm2 = io["wm1"], io["wm1a"], io["wm1b"], io["wm2"]
    offbp, biasy = io["offbp"], io["biasy"]
    jmap, imap, pbase = io["jmap"], io["imap"], io["pbase"]
    y_out = io["y"]

    with tc.tile_pool(name="dram", bufs=1, space="DRAM") as dramp, \
         tc.tile_pool(name="consts", bufs=1) as cstp:
        # debug builds pass these as ExternalOutputs via io
        ox_dram = io.get("dbg_ox") or dramp.tile([2, N], F32)
        cmaps = io.get("dbg_cm") or dramp.tile([4, N], BF16)
        idxw = io.get("dbg_ix") or dramp.tile([1, N], I16)

        offbp_s = cstp.tile([128, 2], F32, tag="offbp")
        nc.sync.dma_start(offbp_s[:], offbp[:])
        biasy_s = cstp.tile([C, 1], F32, tag="biasy")
        nc.sync.dma_start(biasy_s[:], biasy[:])
        pbase_s = cstp.tile([128, 1], F32, tag="pbase")
        nc.sync.dma_start(pbase_s[:], pbase[:])
        wo18_s = cstp.tile([128, 32], BF16, tag="wo18")
        nc.sync.dma_start(
            wo18_s[:], bass.AP(tensor=wo18[:].tensor, offset=wo18[:].offset,
                               ap=[[0, 2], [32, C], [1, 32]]))
        sel18_s = cstp.tile([18, 32], BF16, tag="sel18")
        nc.sync.dma_start(sel18_s[:], sel18[:])

        # ---------------- Phase A: offset head (all bf16) ----------------
        # stage 1: 3 row-results [18, W] packed at PSUM partition bases
        # {0, 32, 64}; tiles indexed so each of the 3 partition groups covers
        # a contiguous row band -> one engine copy per tile, 3 unstack DMAs
        # per block. sel18 [2, 512] results packed the same way.
        # x rows are split across both partition halves of xt (the second
        # half uses the lhsT copy at partition base 64).
        XH = (RA + 2) // 2 + 1  # 34 rows per xt half
        with tc.tile_pool(name="xa", bufs=2) as xap, \
             tc.tile_pool(name="o18", bufs=1) as o18p, \
             tc.tile_pool(name="al", bufs=2) as alp, \
             tc.tile_pool(name="stg", bufs=2) as stgp, \
             tc.tile_pool(name="oxs", bufs=1) as oxsp, \
             tc.tile_pool(name="psA", bufs=4, space="PSUM") as psA, \
             tc.tile_pool(name="psA2", bufs=2, space="PSUM") as psA2:
            cp_i = 0
            for r0 in range(0, H, RA):
                lo = max(0, r0 - 1)
                hi = min(H, r0 + RA + 1)
                nr = hi - lo
                nt = (nr + 2) // 3
                xt = xap.tile([128, XH * W], BF16, tag="xa")
                nc.sync.dma_start(xt[0:C, 0:XH * W],
                                  x_cm[:, lo * W:(lo + XH) * W])
                nc.sync.dma_start(xt[C:2 * C, 0:(nr - XH) * W],
                                  x_cm[:, (lo + XH) * W:hi * W])
                o18s = o18p.tile([18, (RA + 2) * Po], BF16, tag="o18")
                o18v = o18s[:].rearrange("p (r w) -> p r w", w=Po)
                nc.vector.memset(o18v[:, :, 0:1], 0.0)
                nc.vector.memset(o18v[:, :, Po - 1:Po], 0.0)
                if r0 == 0:
                    nc.vector.memset(o18v[:, 0:1, :], 0.0)
                if r0 + RA >= H:
                    nc.vector.memset(o18v[:, RA + 1:RA + 2, :], 0.0)
                stgb = stgp.tile([96, nt * W], BF16, tag="stg")
                for ti in range(nt):
                    ps = psA.tile([96, W], F32, tag="psA")
                    nwr = 0
                    for q in range(3):
                        r = lo + q * nt + ti
                        if r >= hi:
                            continue
                        nwr = q + 1
                        hh = 0 if (r - lo) < XH else 1
                        nc.tensor.matmul(
                            ps[32 * q:32 * q + 32, :],
                            wo18_s[64 * hh:64 * hh + 64, :],
                            xt[64 * hh:64 * hh + 64,
                               (r - lo - XH * hh) * W:
                               (r - lo - XH * hh + 1) * W],
                            start=True, stop=True)
                    np_ = 32 * nwr
                    dst = stgb[0:np_, ti * W:(ti + 1) * W]
                    if cp_i % 2 == 0:
                        nc.vector.tensor_copy(dst, ps[0:np_, :])
                    else:
                        nc.scalar.activation(dst, ps[0:np_, :], AF.Copy,
                                             scale=1.0)
                    cp_i += 1
                for q in range(3):
                    rlo = lo + q * nt
                    rhi = min(hi, rlo + nt)
                    srow = rlo - r0 + 1
                    nc.sync.dma_start(
                        o18v[:, srow:srow + rhi - rlo, 1:W + 1],
                        stgb[32 * q:32 * q + 18, 0:(rhi - rlo) * W])
                al = alp.tile([18, RA * W], BF16, tag="al")
                for t in range(9):
                    dr, ds = t // 3 - 1, t % 3 - 1
                    nc.sync.dma_start(
                        al[2 * t:2 * t + 2, :].rearrange(
                            "p (r w) -> p r w", w=W),
                        o18v[2 * t:2 * t + 2, 1 + dr:1 + dr + RA,
                             1 + ds:1 + ds + W])
                nsg = RA * W // 512  # 32 sel18 segments of 512 px
                ntb = (nsg + 2) // 3
                oxb = oxsp.tile([96, ntb * 512], F32, tag="oxs")
                for ti in range(ntb):
                    ps2 = psA2.tile([96, 512], F32, tag="psA2")
                    nwr = 0
                    for q in range(3):
                        sg = q * ntb + ti
                        if sg >= nsg:
                            continue
                        nwr = q + 1
                        nc.tensor.matmul(
                            ps2[32 * q:32 * q + 32, :], sel18_s[:],
                            al[:, sg * 512:(sg + 1) * 512],
                            start=True, stop=True)
                    np_ = 32 * nwr
                    dst = oxb[0:np_, ti * 512:(ti + 1) * 512]
                    if cp_i % 2 == 0:
                        nc.vector.tensor_copy(dst, ps2[0:np_, :])
                    else:
                        nc.scalar.activation(dst, ps2[0:np_, :], AF.Copy,
                                             scale=1.0)
                    cp_i += 1
                for q in range(3):
                    sgl = q * ntb
                    sgh = min(nsg, sgl + ntb)
                    nc.sync.dma_start(
                        bass.AP(tensor=ox_dram[:].tensor,
                                offset=ox_dram[:].offset + r0 * W + sgl * 512,
                                ap=[[N, 2], [1, (sgh - sgl) * 512]]),
                        oxb[32 * q:32 * q + 2, 0:(sgh - sgl) * 512])

        # ---------------- Coords (compact [128, CF] layout) --------------
        with tc.tile_pool(name="mp", bufs=1) as mp:
            jm = mp.tile([128, CF], F32, tag="jm")
            nc.sync.dma_start(jm[:], jmap[:])
            im = mp.tile([128, CF], F32, tag="im")
            nc.sync.dma_start(im[:], imap[:])

            def coord_chain(row, base_map, clmax):
                """-> (c0f floor-coord f32, wf frac f32) in compact layout."""
                oc = mp.tile([128, CF], F32, tag=f"oc{row}")
                nc.sync.dma_start(
                    oc[:], bass.AP(tensor=ox_dram[:].tensor,
                                   offset=ox_dram[:].offset + row * N,
                                   ap=[[CF, 128], [1, CF]]))
                tn = mp.tile([128, CF], F32, tag=f"tn{row}")
                nc.scalar.activation(tn[:], oc[:], AF.Tanh,
                                     bias=offbp_s[:, row:row + 1], scale=1.0)
                ic = mp.tile([128, CF], F32, tag=f"ic{row}")
                nc.vector.scalar_tensor_tensor(ic[:], tn[:], 2.0, base_map[:],
                                               ALU.mult, ALU.add)
                nc.vector.tensor_scalar(ic[:], ic[:], 0.0, clmax,
                                        ALU.max, ALU.min)
                i32t = mp.tile([128, CF], I32, tag=f"i32{row}")
                nc.vector.tensor_copy(i32t[:], ic[:])
                c0f = mp.tile([128, CF], F32, tag=f"c0f{row}")
                nc.vector.tensor_copy(c0f[:], i32t[:])
                wf = mp.tile([128, CF], F32, tag=f"wf{row}")
                nc.vector.tensor_tensor(wf[:], ic[:], c0f[:], ALU.subtract)
                # hw f32->i32 rounds to nearest; correct to floor
                msk = mp.tile([128, CF], F32, tag=f"msk{row}")
                nc.vector.tensor_scalar(msk[:], wf[:], 0.0, None, ALU.is_lt)
                nc.vector.tensor_tensor(c0f[:], c0f[:], msk[:], ALU.subtract)
                nc.vector.tensor_tensor(wf[:], ic[:], c0f[:], ALU.subtract)
                return c0f, wf

            x0f, wxf = coord_chain(0, jm, CLX)
            y0f, wyf = coord_chain(1, im, CLY)
            for nm, t in (("dbg_x0f", x0f), ("dbg_wxf", wxf),
                          ("dbg_y0f", y0f), ("dbg_wyf", wyf)):
                if io.get(nm) is not None:
                    nc.sync.dma_start(io[nm][:], t[:])

            vx0 = mp.tile([128, CF], F32, tag="vx0")
            nc.vector.tensor_scalar(vx0[:], wxf[:], -1.0, 1.0,
                                    ALU.mult, ALU.add)
            vy0 = mp.tile([128, CF], F32, tag="vy0")
            nc.vector.tensor_scalar(vy0[:], wyf[:], -1.0, 1.0,
                                    ALU.mult, ALU.add)
            cmt = mp.tile([128, 4, CF], BF16, tag="cmt")
            nc.vector.tensor_tensor(cmt[:, 0, :], vy0[:], vx0[:], ALU.mult)
            nc.vector.tensor_tensor(cmt[:, 1, :], vy0[:], wxf[:], ALU.mult)
            nc.vector.tensor_tensor(cmt[:, 2, :], wyf[:], vx0[:], ALU.mult)
            nc.vector.tensor_tensor(cmt[:, 3, :], wyf[:], wxf[:], ALU.mult)
            nc.sync.dma_start(
                bass.AP(tensor=cmaps[:].tensor, offset=cmaps[:].offset,
                        ap=[[CF, 128], [N, 4], [1, CF]]),
                cmt[:])

            idxf = mp.tile([128, CF], F32, tag="idxf")
            nc.vector.scalar_tensor_tensor(idxf[:], y0f[:], float(W), x0f[:],
                                           ALU.mult, ALU.add)
            nc.vector.tensor_scalar(idxf[:], idxf[:], pbase_s[:], None,
                                    ALU.subtract)
            # i16 convert + in-partition (a,b)->(b,a) shuffle so the DRAM
            # write below is stride-1-innermost on both sides.
            # block k (4096 px = partitions 8k..8k+8), local pixel
            # m = q*512 + c, c = 16a+b  ->  dram pos k*4096 + 256*b + 32*q + a
            iiw = mp.tile([128, CF], I16, tag="iiw")
            nc.vector.tensor_copy(
                iiw[:].rearrange("p (b a) -> p b a", a=32),
                idxf[:].rearrange("p (a b) -> p b a", b=16))
            for k in range(N // MB):
                src = iiw[8 * k:8 * k + 8, :].rearrange(
                    "p (b a) -> p b a", a=32)
                nc.sync.dma_start(
                    bass.AP(tensor=idxw[:].tensor,
                            offset=idxw[:].offset + k * MB,
                            ap=[[32, 8], [256, 16], [1, 32]]),
                    src)

        # ---------------- Phase BC: gather + combine + conv --------------
        with tc.tile_pool(name="w2", bufs=1) as w2p, \
             tc.tile_pool(name="wc", bufs=1) as wc:
            W2 = w2p.tile([128, RING * W], BF16, tag="W2")
            wm1_s = wc.tile([C, 3 * C], BF16, tag="wm1")
            nc.sync.dma_start(wm1_s[:], wm1[:])
            wm1a_s = wc.tile([C, 3 * C], BF16, tag="wm1a")
            nc.sync.dma_start(wm1a_s[:], wm1a[:])
            wm1b_s = wc.tile([C, 3 * C], BF16, tag="wm1b")
            nc.sync.dma_start(wm1b_s[:], wm1b[:])
            wm2_s = wc.tile([128, 3 * C], BF16, tag="wm2")
            nc.sync.dma_start(wm2_s[:], wm2[:])

            with tc.tile_pool(name="gb", bufs=2) as gbp, \
                 tc.tile_pool(name="cwb", bufs=2) as cwp, \
                 tc.tile_pool(name="ixb", bufs=2) as ixp, \
                 tc.tile_pool(name="th", bufs=2) as thp, \
                 tc.tile_pool(name="yb", bufs=2) as ybp, \
                 tc.tile_pool(name="psC", bufs=2, space="PSUM") as psC:

                def conv_rows(rlo, rhi):
                    seg = {-1: (DIL, W, -DIL), 0: (0, W, 0),
                           1: (0, W - DIL, DIL)}
                    for r8 in range(rlo, rhi, 8):
                        ps = psC.tile([C, 8 * W], F32, tag="psC")
                        for r in range(r8, r8 + 8):
                            po = (r - r8) * W
                            mms = []
                            for ds in (0, -1, 1):
                                olo, ohi, dsoff = seg[ds]
                                gcol = (ds + 1) * C
                                base = (r % RING) * W
                                mms.append(
                                    (ps[:, po + olo:po + ohi],
                                     wm1_s[:, gcol:gcol + C],
                                     W2[0:64, base + olo + dsoff:
                                        base + ohi + dsoff]))
                                if DIL <= r < H - DIL:
                                    b2 = ((r - DIL) % RING) * W
                                    mms.append(
                                        (ps[:, po + olo:po + ohi],
                                         wm2_s[:, gcol:gcol + C],
                                         W2[:, b2 + olo + dsoff:
                                            b2 + ohi + dsoff]))
                                elif r < DIL:
                                    b2 = ((r + DIL) % RING) * W
                                    mms.append(
                                        (ps[:, po + olo:po + ohi],
                                         wm1b_s[:, gcol:gcol + C],
                                         W2[0:64, b2 + olo + dsoff:
                                            b2 + ohi + dsoff]))
                                else:
                                    b2 = ((r - DIL) % RING) * W
                                    mms.append(
                                        (ps[:, po + olo:po + ohi],
                                         wm1a_s[:, gcol:gcol + C],
                                         W2[0:64, b2 + olo + dsoff:
                                            b2 + ohi + dsoff]))
                            for mi, (o, l, rr) in enumerate(mms):
                                nc.tensor.matmul(o, l, rr, start=(mi == 0),
                                                 stop=(mi == len(mms) - 1))
                        yb = ybp.tile([C, 8 * W], BF16, tag="yb")
                        nc.scalar.activation(yb[:], ps[:], AF.Relu,
                                             bias=biasy_s[:], scale=1.0)
                        nc.sync.dma_start(y_out[:, r8 * W:(r8 + 8) * W],
                                          yb[:])

                for k in range(N // MB):
                    r0 = k * RB
                    base_px = max(0, r0 - 2) * W
                    ixt = ixp.tile([128, MB // 16], I16, tag="ix")
                    nc.sync.dma_start(
                        ixt[:], bass.AP(tensor=idxw[:].tensor,
                                        offset=idxw[:].offset + k * MB,
                                        ap=[[0, 8], [MB // 16, 16],
                                            [1, MB // 16]]))
                    g = gbp.tile([128, 2, MB], BF16, tag="g")
                    nc.gpsimd.dma_gather(
                        g[:], bass.AP(tensor=x_pm4[:].tensor,
                                      offset=x_pm4[:].offset + base_px * 4 * C,
                                      ap=[[4 * C, N - base_px], [1, 4 * C]]),
                        ixt[:], MB, MB, 4 * C, transpose=True,
                        single_packet=False)
                    cw = cwp.tile([128, 2, MB], BF16, tag="cw")
                    for gi in range(2):
                        nc.sync.dma_start(
                            cw[:, gi, :],
                            bass.AP(tensor=cmaps[:].tensor,
                                    offset=cmaps[:].offset + 2 * gi * N
                                    + r0 * W,
                                    ap=[[N, 2], [0, 64], [1, MB]]))
                    g0 = g[:, 0, :]
                    g1 = g[:, 1, :]
                    nc.vector.tensor_tensor(g0, g0, cw[:, 0, :], ALU.mult)
                    nc.vector.tensor_tensor(g1, g1, cw[:, 1, :], ALU.mult)
                    nc.vector.tensor_tensor(g0, g0, g1, ALU.add)
                    th = thp.tile([64, MB], BF16, tag="th")
                    nc.scalar.copy(th[:], g0[64:128])
                    slot = (r0 % RING) * W
                    nc.vector.tensor_tensor(
                        W2[0:64, slot:slot + MB], g0[0:64], th[:],
                        ALU.add)
                    # fill partitions 64:128 (row +24 copies) for slot-rows
                    # [r0-24, r0-8) in two 8-row pieces
                    for s in (r0 - 24, r0 - 16):
                        if s < 0:
                            continue
                        dsl = (s % RING) * W
                        ssl = ((s + 24) % RING) * W
                        nc.vector.tensor_copy(
                            W2[64:128, dsl:dsl + 8 * W],
                            W2[0:64, ssl:ssl + 8 * W])
                    # 32-row conv chunks after even blocks (fewer PE
                    # restarts -> less p-state ramp penalty)
                    if k >= 2 and k % 2 == 0:
                        conv_rows(16 * k - 32, 16 * k)
                conv_rows(H - 2 * RB, H)


_NC_CACHE = {}


def build_io(nc):
    io = {}
    for name, shape, dt in IN_SPECS:
        mdt = BF16 if dt is NPBF16 else F32
        io[name] = nc.dram_tensor(name, list(shape), mdt,
                                  kind="ExternalInput").ap()
    io["y"] = nc.dram_tensor("y", [C, N], BF16, kind="ExternalOutput").ap()
    return io


def build_nc():
    if "nc" in _NC_CACHE:
        return _NC_CACHE["nc"]
    nc = bacc.Bacc("TRN2", target_bir_lowering=False, debug=False,
                   num_devices=N_CORES)
    io = build_io(nc)
    with tile.TileContext(nc) as tc:
        emit(tc, io, H, W)
    nc.compile()
    _NC_CACHE["nc"] = nc
    return nc


def kernel(x, offset_w, offset_b, conv_w, bn_gamma, bn_beta, bn_mean, bn_var):
    x = np.asarray(x, np.float32)
    offset_w = np.asarray(offset_w, np.float32)
    offset_b = np.asarray(offset_b, np.float32)
    conv_w = np.asarray(conv_w, np.float32)
    bn_gamma = np.asarray(bn_gamma, np.float32)
    bn_beta = np.asarray(bn_beta, np.float32)
    bn_mean = np.asarray(bn_mean, np.float32)
    bn_var = np.asarray(bn_var, np.float32)
    B = x.shape[0]
    nc = build_nc()
    shared = prep_shared(offset_w, offset_b, conv_w, bn_gamma, bn_beta,
                         bn_mean, bn_var)
    in_maps = []
    for b in range(B):
        m = dict(shared)
        m.update(prep_x(x[b]))
        in_maps.append(m)
    res = bass_utils.run_bass_kernel_spmd(nc, in_maps,
                                          core_ids=list(range(B)))
    out = np.stack([
        np.asarray(res.results[b]["y"], dtype=np.float32).reshape(C, H, W)
        for b in range(B)])
    return out


# revision 61
# speedup vs baseline: 2.3780x; 1.0949x over previous
"""Trainium2 Bass kernel for nn_DeformLikeASPPConv (8-core data parallel).

Self-contained: kernel(**inputs) takes the full-batch inputs and returns the
full output. One sample per NeuronCore. See emit() for the device pipeline.

Fully fused pipeline (per core, one sample [64, 256, 256]):
  per 32-row A-block: offset-head 3x3 conv (18-partial trick, bf16, results
  packed 3-wide across PSUM partition groups), tanh + sampling coords +
  compound bilinear weights + i16 gather indices in a block-local compact
  [128, 64] layout; then per 16-row BC block: dma_gather of the 4 bilinear
  neighbors -> compound-weight combine (DVE) -> warped rows into a 64-row
  ring -> dilated 3x3 conv (PE, 32-row chunks) + BN/ReLU -> bf16 output.
All stages overlap across blocks via tile-pool double buffering.
"""
import sys
if "/opt/trn_rl_repo" not in sys.path:
    sys.path.insert(0, "/opt/trn_rl_repo")
import numpy as np
import ml_dtypes
import concourse.bass as bass
import concourse.bacc as bacc
import concourse.tile as tile
import concourse.mybir as mybir
from concourse import bass_utils

N_CORES = 8
H, W = 256, 256
N = H * W

NPBF16 = ml_dtypes.bfloat16
C = 64
DIL = 12
BN_EPS = 1e-5

RA = 64          # A-block rows
RB = 16          # BC-block rows
MB = RB * W      # 4096 pixels per BC block
RING = 96        # warped ring rows
NA = RA * W      # 8192 pixels per A-block
CA = NA // 128   # 64 compact cols per A-block
NAB = H // RA    # 8 A-blocks
XH = (RA + 2) // 2 + 1  # 18 rows in first xt half


def prep_core_inputs(x, offset_w, offset_b, conv_w, bn_gamma, bn_beta,
                     bn_mean, bn_var):
    """x: [C, H, W] fp32 one sample -> dict of kernel inputs."""
    base = prep_shared(offset_w, offset_b, conv_w, bn_gamma, bn_beta,
                       bn_mean, bn_var)
    base.update(prep_x(x))
    return base


def prep_x(x):
    x_cm = x.reshape(C, N).astype(NPBF16)
    pm = np.ascontiguousarray(x.reshape(C, N).T).astype(NPBF16)  # [N, C]
    p = np.arange(N)
    x_pm4 = np.concatenate([
        pm[np.minimum(p + d, N - 1)] for d in (0, 1, W, W + 1)],
        axis=1)  # [N, 4C]
    return {"x_cm": x_cm, "x_pm4": x_pm4}


def prep_shared(offset_w, offset_b, conv_w, bn_gamma, bn_beta, bn_mean,
                bn_var):
    wo18 = np.zeros((C, 32), np.float32)  # cols 18:32 zero-padded so the
    for t in range(9):                    # packed matmuls fill whole
        r, s = t // 3, t % 3              # 32-partition PSUM groups
        for o in range(2):
            wo18[:, 2 * t + o] = offset_w[o, :, r, s]
    sel18 = np.zeros((18, 32), np.float32)
    for t in range(9):
        for o in range(2):
            sel18[2 * t + o, o] = 1.0
    inv = (bn_gamma / np.sqrt(bn_var + BN_EPS)).astype(np.float32)
    wmf = conv_w * inv[:, None, None, None]  # [Cout, Cin, 3, 3]
    wm1 = np.zeros((C, 3 * C), np.float32)
    wm1a = np.zeros((C, 3 * C), np.float32)
    wm1b = np.zeros((C, 3 * C), np.float32)
    wm2 = np.zeros((2 * C, 3 * C), np.float32)
    for gs in range(3):  # gcol = (ds+1)*C with ds = gs-1
        wm1[:, gs * C:(gs + 1) * C] = wmf[:, :, 1, gs].T
        wm1a[:, gs * C:(gs + 1) * C] = wmf[:, :, 0, gs].T
        wm1b[:, gs * C:(gs + 1) * C] = wmf[:, :, 2, gs].T
        wm2[0:C, gs * C:(gs + 1) * C] = wmf[:, :, 0, gs].T
        wm2[C:2 * C, gs * C:(gs + 1) * C] = wmf[:, :, 2, gs].T
    biasy = (bn_beta - bn_mean * inv).astype(np.float32).reshape(C, 1)
    # block-local compact maps: A-block a, partition p, col c ->
    # global pixel 8192*a + 64*p + c
    parts = np.arange(128)[:, None]
    cols = np.arange(CA)[None, :]
    jm2 = np.zeros((128, NAB * CA), np.float32)
    im2 = np.zeros((128, NAB * CA), np.float32)
    pb2 = np.zeros((128, NAB), np.float32)
    for a in range(NAB):
        g = NA * a + CA * parts + cols
        jm2[:, a * CA:(a + 1) * CA] = g % W
        im2[:, a * CA:(a + 1) * CA] = g // W
        blk = (RA // RB) * a + parts[:, 0] // (128 * RB // RA)
        pb2[:, a] = np.maximum(0, RB * blk - 2) * W
    return {
        "wo18": wo18.astype(NPBF16),
        "sel18": sel18.astype(NPBF16),
        "wm1": wm1.astype(NPBF16),
        "wm1a": wm1a.astype(NPBF16),
        "wm1b": wm1b.astype(NPBF16),
        "wm2": wm2.astype(NPBF16),
        "offbp": np.broadcast_to(offset_b.astype(np.float32)[None, :],
                                 (128, 2)).copy(),
        "biasy": biasy,
        "jmap": jm2,
        "imap": im2,
        "pbase": pb2,
    }


IN_SPECS = [
    ("x_cm", (C, N), NPBF16),
    ("x_pm4", (N, 4 * C), NPBF16),
    ("wo18", (C, 32), NPBF16),
    ("sel18", (18, 32), NPBF16),
    ("wm1", (C, 3 * C), NPBF16),
    ("wm1a", (C, 3 * C), NPBF16),
    ("wm1b", (C, 3 * C), NPBF16),
    ("wm2", (2 * C, 3 * C), NPBF16),
    ("offbp", (128, 2), np.float32),
    ("biasy", (C, 1), np.float32),
    ("jmap", (128, NAB * CA), np.float32),
    ("imap", (128, NAB * CA), np.float32),
    ("pbase", (128, NAB), np.float32),
]

F32 = mybir.dt.float32
BF16 = mybir.dt.bfloat16
I16 = mybir.dt.int16
I32 = mybir.dt.int32
ALU = mybir.AluOpType
AF = mybir.ActivationFunctionType

CLX = (W - 2) + 0.99609375
CLY = (H - 2) + 0.99609375


def emit(tc, io, H_, W_):
    nc = tc.nc
    Po = W + 2

    x_cm, x_pm4 = io["x_cm"], io["x_pm4"]
    wo18, sel18 = io["wo18"], io["sel18"]
    wm1, wm1a, wm1b, wm2 = io["wm1"], io["wm1a"], io["wm1b"], io["wm2"]
    offbp, biasy = io["offbp"], io["biasy"]
    jmap, imap, pbase = io["jmap"], io["imap"], io["pbase"]
    y_out = io["y"]

    with tc.tile_pool(name="dram", bufs=1, space="DRAM") as dramp, \
         tc.tile_pool(name="consts", bufs=1) as cstp:
        # debug builds pass these as ExternalOutputs via io
        ox_dram = io.get("dbg_ox") or dramp.tile([2, N], F32)
        cmaps = io.get("dbg_cm") or dramp.tile([4, N], BF16)
        idxw = io.get("dbg_ix") or dramp.tile([1, N], I16)

        offbp_s = cstp.tile([128, 2], F32, tag="offbp")
        nc.sync.dma_start(offbp_s[:], offbp[:])
        biasy_s = cstp.tile([C, 1], F32, tag="biasy")
        nc.sync.dma_start(biasy_s[:], biasy[:])
        pb2_s = cstp.tile([128, NAB], F32, tag="pbase")
        nc.sync.dma_start(pb2_s[:], pbase[:])
        wo18_s = cstp.tile([128, 32], BF16, tag="wo18")
        nc.sync.dma_start(
            wo18_s[:], bass.AP(tensor=wo18[:].tensor, offset=wo18[:].offset,
                               ap=[[0, 2], [32, C], [1, 32]]))
        sel18_s = cstp.tile([18, 32], BF16, tag="sel18")
        nc.sync.dma_start(sel18_s[:], sel18[:])
        jm2 = cstp.tile([128, NAB * CA], F32, tag="jm2")
        nc.sync.dma_start(jm2[:], jmap[:])
        im2 = cstp.tile([128, NAB * CA], F32, tag="im2")
        nc.sync.dma_start(im2[:], imap[:])
        wm1_s = cstp.tile([C, 3 * C], BF16, tag="wm1")
        nc.sync.dma_start(wm1_s[:], wm1[:])
        wm1a_s = cstp.tile([C, 3 * C], BF16, tag="wm1a")
        nc.sync.dma_start(wm1a_s[:], wm1a[:])
        wm1b_s = cstp.tile([C, 3 * C], BF16, tag="wm1b")
        nc.sync.dma_start(wm1b_s[:], wm1b[:])
        wm2_s = cstp.tile([128, 3 * C], BF16, tag="wm2")
        nc.sync.dma_start(wm2_s[:], wm2[:])

        with tc.tile_pool(name="xa", bufs=2) as xap, \
             tc.tile_pool(name="o18", bufs=1) as o18p, \
             tc.tile_pool(name="al", bufs=1) as alp, \
             tc.tile_pool(name="stg", bufs=2) as stgp, \
             tc.tile_pool(name="oxs", bufs=1) as oxsp, \
             tc.tile_pool(name="mp", bufs=2) as mp, \
             tc.tile_pool(name="w2", bufs=1) as w2p, \
             tc.tile_pool(name="gb", bufs=2) as gbp, \
             tc.tile_pool(name="cwb", bufs=2) as cwp, \
             tc.tile_pool(name="ixb", bufs=2) as ixp, \
             tc.tile_pool(name="yb", bufs=2) as ybp, \
             tc.tile_pool(name="psA", bufs=2, space="PSUM") as psA, \
             tc.tile_pool(name="psA2", bufs=2, space="PSUM") as psA2, \
             tc.tile_pool(name="psC", bufs=2, space="PSUM") as psC:
            W2 = w2p.tile([128, RING * W], BF16, tag="W2")

            cp = {"i": 0}
            albuf = {}

            def pcopy(dst, src):
                if cp["i"] % 2 == 0:
                    nc.vector.tensor_copy(dst, src)
                else:
                    nc.scalar.activation(dst, src, AF.Copy, scale=1.0)
                cp["i"] += 1

            xtbuf = {}

            def load_x(a):
                lo = max(0, RA * a - 1)
                hi = min(H, RA * a + RA + 1)
                nr = hi - lo
                xt = xap.tile([128, XH * W], BF16, tag="xa")
                nc.sync.dma_start(xt[0:C, 0:XH * W],
                                  x_cm[:, lo * W:(lo + XH) * W])
                nc.sync.dma_start(xt[C:2 * C, 0:(nr - XH) * W],
                                  x_cm[:, (lo + XH) * W:hi * W])
                xtbuf[a] = xt

            def do_stage1(a):
                r0 = RA * a
                lo = max(0, r0 - 1)
                hi = min(H, r0 + RA + 1)
                nr = hi - lo
                nt = (nr + 2) // 3
                xt = xtbuf.pop(a)
                o18s = o18p.tile([18, (RA + 2) * Po], BF16, tag="o18")
                o18v = o18s[:].rearrange("p (r w) -> p r w", w=Po)
                nc.vector.memset(o18v[:, :, 0:1], 0.0)
                nc.vector.memset(o18v[:, :, Po - 1:Po], 0.0)
                if r0 == 0:
                    nc.vector.memset(o18v[:, 0:1, :], 0.0)
                if r0 + RA >= H:
                    nc.vector.memset(o18v[:, RA + 1:RA + 2, :], 0.0)
                stgb = stgp.tile([96, nt * W], BF16, tag="stg")
                for ti in range(nt):
                    ps = psA.tile([96, W], F32, tag="psA")
                    nwr = 0
                    for q in range(3):
                        r = lo + q * nt + ti
                        if r >= hi:
                            continue
                        nwr = q + 1
                        hh = 0 if (r - lo) < XH else 1
                        nc.tensor.matmul(
                            ps[32 * q:32 * q + 32, :],
                            wo18_s[64 * hh:64 * hh + 64, :],
                            xt[64 * hh:64 * hh + 64,
                               (r - lo - XH * hh) * W:
                               (r - lo - XH * hh + 1) * W],
                            start=True, stop=True)
                    pcopy(stgb[0:32 * nwr, ti * W:(ti + 1) * W],
                          ps[0:32 * nwr, :])
                for q in range(3):
                    rlo = lo + q * nt
                    rhi = min(hi, rlo + nt)
                    srow = rlo - r0 + 1
                    nc.gpsimd.dma_start(
                        o18v[:, srow:srow + rhi - rlo, 1:W + 1],
                        stgb[32 * q:32 * q + 18, 0:(rhi - rlo) * W])
                al = alp.tile([18, RA * W], BF16, tag="al")
                for t in range(9):
                    dr, ds = t // 3 - 1, t % 3 - 1
                    nc.gpsimd.dma_start(
                        al[2 * t:2 * t + 2, :].rearrange(
                            "p (r w) -> p r w", w=W),
                        o18v[2 * t:2 * t + 2, 1 + dr:1 + dr + RA,
                             1 + ds:1 + ds + W])
                albuf[a] = al

            def do_sel18(a):
                r0 = RA * a
                al = albuf.pop(a)
                nsg = RA * W // 512  # 16 sel18 segments of 512 px
                ntb = (nsg + 2) // 3
                oxb = oxsp.tile([96, ntb * 512], F32, tag="oxs")
                for ti in range(ntb):
                    ps2 = psA2.tile([96, 512], F32, tag="psA2")
                    nwr = 0
                    for q in range(3):
                        sg = q * ntb + ti
                        if sg >= nsg:
                            continue
                        nwr = q + 1
                        nc.tensor.matmul(
                            ps2[32 * q:32 * q + 32, :], sel18_s[:],
                            al[:, sg * 512:(sg + 1) * 512],
                            start=True, stop=True)
                    pcopy(oxb[0:32 * nwr, ti * 512:(ti + 1) * 512],
                          ps2[0:32 * nwr, :])
                for q in range(3):
                    sgl = q * ntb
                    sgh = min(nsg, sgl + ntb)
                    nc.sync.dma_start(
                        bass.AP(tensor=ox_dram[:].tensor,
                                offset=ox_dram[:].offset + r0 * W + sgl * 512,
                                ap=[[N, 2], [1, (sgh - sgl) * 512]]),
                        oxb[32 * q:32 * q + 2, 0:(sgh - sgl) * 512])

            def do_coords(a):
                def coord_chain(row, base_map, clmax):
                    oc = mp.tile([128, CA], F32, tag=f"oc{row}")
                    nc.sync.dma_start(
                        oc[:], bass.AP(tensor=ox_dram[:].tensor,
                                       offset=ox_dram[:].offset + row * N
                                       + a * NA,
                                       ap=[[CA, 128], [1, CA]]))
                    tn = mp.tile([128, CA], F32, tag=f"tn{row}")
                    nc.scalar.activation(tn[:], oc[:], AF.Tanh,
                                         bias=offbp_s[:, row:row + 1],
                                         scale=1.0)
                    ic = mp.tile([128, CA], F32, tag=f"ic{row}")
                    nc.vector.scalar_tensor_tensor(
                        ic[:], tn[:], 2.0, base_map[:, a * CA:(a + 1) * CA],
                        ALU.mult, ALU.add)
                    nc.vector.tensor_scalar(ic[:], ic[:], 0.0, clmax,
                                            ALU.max, ALU.min)
                    i32t = mp.tile([128, CA], I32, tag=f"i32{row}")
                    nc.vector.tensor_copy(i32t[:], ic[:])
                    c0f = mp.tile([128, CA], F32, tag=f"c0f{row}")
                    nc.vector.tensor_copy(c0f[:], i32t[:])
                    wf = mp.tile([128, CA], F32, tag=f"wf{row}")
                    nc.vector.tensor_tensor(wf[:], ic[:], c0f[:],
                                            ALU.subtract)
                    # hw f32->i32 rounds to nearest; correct to floor
                    msk = mp.tile([128, CA], F32, tag=f"msk{row}")
                    nc.vector.tensor_scalar(msk[:], wf[:], 0.0, None,
                                            ALU.is_lt)
                    nc.vector.tensor_tensor(c0f[:], c0f[:], msk[:],
                                            ALU.subtract)
                    nc.vector.tensor_tensor(wf[:], ic[:], c0f[:],
                                            ALU.subtract)
                    return c0f, wf

                x0f, wxf = coord_chain(0, jm2, CLX)
                y0f, wyf = coord_chain(1, im2, CLY)
                for nm, t in (("dbg_x0f", x0f), ("dbg_wxf", wxf),
                              ("dbg_y0f", y0f), ("dbg_wyf", wyf)):
                    if io.get(nm) is not None:
                        nc.sync.dma_start(io[nm][:, a * CA:(a + 1) * CA],
                                          t[:])
                vx0 = mp.tile([128, CA], F32, tag="vx0")
                nc.vector.tensor_scalar(vx0[:], wxf[:], -1.0, 1.0,
                                        ALU.mult, ALU.add)
                vy0 = mp.tile([128, CA], F32, tag="vy0")
                nc.vector.tensor_scalar(vy0[:], wyf[:], -1.0, 1.0,
                                        ALU.mult, ALU.add)
                cmt = mp.tile([128, 4, CA], BF16, tag="cmt")
                nc.vector.tensor_tensor(cmt[:, 0, :], vy0[:], vx0[:],
                                        ALU.mult)
                nc.vector.tensor_tensor(cmt[:, 1, :], vy0[:], wxf[:],
                                        ALU.mult)
                nc.vector.tensor_tensor(cmt[:, 2, :], wyf[:], vx0[:],
                                        ALU.mult)
                nc.vector.tensor_tensor(cmt[:, 3, :], wyf[:], wxf[:],
                                        ALU.mult)
                nc.sync.dma_start(
                    bass.AP(tensor=cmaps[:].tensor,
                            offset=cmaps[:].offset + a * NA,
                            ap=[[CA, 128], [N, 4], [1, CA]]),
                    cmt[:])
                idxf = mp.tile([128, CA], F32, tag="idxf")
                nc.vector.scalar_tensor_tensor(idxf[:], y0f[:], float(W),
                                               x0f[:], ALU.mult, ALU.add)
                nc.vector.tensor_scalar(idxf[:], idxf[:], pb2_s[:, a:a + 1],
                                        None, ALU.subtract)
                # i16 convert + in-partition (a',b)->(b,a') shuffle; BC
                # block k covers partitions [PPB*j, PPB*(j+1)), local pixel
                # m = (p%PPB)*CA + c, c = 16a'+b  ->  dram pos
                # k*4096 + 256*b + (CA//16)*(p%PPB) + a'
                KPA = RA // RB
                PPB = 128 // KPA
                A2 = CA // 16
                iiw = mp.tile([128, CA], I16, tag="iiw")
                nc.vector.tensor_copy(
                    iiw[:].rearrange("p (b a2) -> p b a2", a2=A2),
                    idxf[:].rearrange("p (a2 b) -> p b a2", b=16))
                for j in range(KPA):
                    k = KPA * a + j
                    nc.sync.dma_start(
                        bass.AP(tensor=idxw[:].tensor,
                                offset=idxw[:].offset + k * MB,
                                ap=[[A2, PPB], [256, 16], [1, A2]]),
                        iiw[PPB * j:PPB * (j + 1), :].rearrange(
                            "p (b a2) -> p b a2", a2=A2))

            # software pipeline: stage1(a) | sel18(a-1) | coords(a-2) |
            # BC blocks of A-block a-3 -- each stage's inputs were produced
            # a full iteration earlier, so no in-order engine stream stalls.
            load_x(0)
            for a in range(NAB + 1):
                if a + 1 < NAB:
                    load_x(a + 1)
                if a < NAB:
                    do_stage1(a)
                if a >= 1:
                    do_sel18(a - 1)
                    do_coords(a - 1)
        # ---------------- Phase BC: gather + combine + conv --------------
        with tc.tile_pool(name="w2", bufs=1) as w2p, \
             tc.tile_pool(name="gb", bufs=5) as gbp, \
             tc.tile_pool(name="cwb", bufs=4) as cwp, \
             tc.tile_pool(name="ixb", bufs=4) as ixp, \
             tc.tile_pool(name="yb", bufs=2) as ybp, \
             tc.tile_pool(name="psC", bufs=4, space="PSUM") as psC:
            W2 = w2p.tile([128, RING * W], BF16, tag="W2")

            def conv_rows(rlo, rhi):
                seg = {-1: (DIL, W, -DIL), 0: (0, W, 0),
                       1: (0, W - DIL, DIL)}
                for r8 in range(rlo, rhi, 8):
                    yb = ybp.tile([C, 8 * W], BF16, tag="yb")
                    for half in range(2):
                        ps = psC.tile([C, 4 * W], F32, tag="psC")
                        for r in range(r8 + 4 * half, r8 + 4 * half + 4):
                            po = (r - r8 - 4 * half) * W
                            mms = []
                            for ds in (0, -1, 1):
                                olo, ohi, dsoff = seg[ds]
                                gcol = (ds + 1) * C
                                base = (r % RING) * W
                                mms.append(
                                    (ps[:, po + olo:po + ohi],
                                     wm1_s[:, gcol:gcol + C],
                                     W2[0:64, base + olo + dsoff:
                                        base + ohi + dsoff]))
                                if DIL <= r < H - DIL:
                                    b2 = ((r - DIL) % RING) * W
                                    mms.append(
                                        (ps[:, po + olo:po + ohi],
                                         wm2_s[:, gcol:gcol + C],
                                         W2[:, b2 + olo + dsoff:
                                            b2 + ohi + dsoff]))
                                elif r < DIL:
                                    b2 = ((r + DIL) % RING) * W
                                    mms.append(
                                        (ps[:, po + olo:po + ohi],
                                         wm1b_s[:, gcol:gcol + C],
                                         W2[0:64, b2 + olo + dsoff:
                                            b2 + ohi + dsoff]))
                                else:
                                    b2 = ((r - DIL) % RING) * W
                                    mms.append(
                                        (ps[:, po + olo:po + ohi],
                                         wm1a_s[:, gcol:gcol + C],
                                         W2[0:64, b2 + olo + dsoff:
                                            b2 + ohi + dsoff]))
                            for mi, (o, l, rr) in enumerate(mms):
                                nc.tensor.matmul(o, l, rr, start=(mi == 0),
                                                 stop=(mi == len(mms) - 1))
                        nc.scalar.activation(
                            yb[:, half * 4 * W:(half + 1) * 4 * W], ps[:],
                            AF.Relu, bias=biasy_s[:], scale=1.0)
                    nc.sync.dma_start(y_out[:, r8 * W:(r8 + 8) * W], yb[:])

            gbuf = {}

            def emit_bc_fetch(k):
                rb0 = k * RB
                base_px = max(0, rb0 - 2) * W
                ixt = ixp.tile([128, MB // 16], I16, tag="ix")
                nc.gpsimd.dma_start(
                    ixt[:], bass.AP(tensor=idxw[:].tensor,
                                    offset=idxw[:].offset + k * MB,
                                    ap=[[0, 8], [MB // 16, 16],
                                        [1, MB // 16]]))
                g = gbp.tile([128, 2, MB], BF16, tag="g")
                nc.gpsimd.dma_gather(
                    g[:], bass.AP(tensor=x_pm4[:].tensor,
                                  offset=x_pm4[:].offset + base_px * 4 * C,
                                  ap=[[4 * C, N - base_px], [1, 4 * C]]),
                    ixt[:], MB, MB, 4 * C, transpose=True,
                    single_packet=False)
                cw = cwp.tile([128, 2, MB], BF16, tag="cw")
                for gi in range(2):
                    nc.gpsimd.dma_start(
                        cw[:, gi, :],
                        bass.AP(tensor=cmaps[:].tensor,
                                offset=cmaps[:].offset + 2 * gi * N
                                + rb0 * W,
                                ap=[[N, 2], [0, 64], [1, MB]]))
                gbuf[k] = (g, cw)

            def emit_bc_compute(k):
                rb0 = k * RB
                g, cw = gbuf.pop(k)
                g0 = g[:, 0, :]
                g1 = g[:, 1, :]
                nc.vector.tensor_tensor(g0, g0, cw[:, 0, :], ALU.mult)
                nc.vector.tensor_tensor(g1, g1, cw[:, 1, :], ALU.mult)
                nc.vector.tensor_tensor(g0, g0, g1, ALU.add)
                slot = (rb0 % RING) * W
                th = g[0:64, 1, :]  # g1 is dead after the add; reuse as the
                nc.vector.tensor_copy(th, g0[64:128])  # base-shift staging
                nc.vector.tensor_tensor(
                    W2[0:64, slot:slot + MB], g0[0:64], th, ALU.add)
                # fill partitions 64:128 (row +24 copies) for slot-rows
                # [rb0-24, rb0-8) in two 8-row pieces
                for s in (rb0 - 24, rb0 - 16):
                    if s < 0:
                        continue
                    dsl = (s % RING) * W
                    ssl = ((s + 24) % RING) * W
                    nc.vector.tensor_copy(
                        W2[64:128, dsl:dsl + 8 * W],
                        W2[0:64, ssl:ssl + 8 * W])
                # conv chunks: small lag-1 chunks during the BC ramp (PE is
                # idle waiting for the first combines anyway), then 32-row
                # chunks one block behind the combines
                if 1 <= k <= 6:
                    conv_rows(16 * (k - 1), 16 * k)
                elif k in (9, 11, 13, 15):
                    conv_rows(16 * k - 48, 16 * k - 16)


            for k in range(N // MB):
                emit_bc_fetch(k)
                emit_bc_compute(k)
            conv_rows(H - 32, H)


_NC_CACHE = {}


def build_io(nc):
    io = {}
    for name, shape, dt in IN_SPECS:
        mdt = BF16 if dt is NPBF16 else F32
        io[name] = nc.dram_tensor(name, list(shape), mdt,
                                  kind="ExternalInput").ap()
    io["y"] = nc.dram_tensor("y", [C, N], BF16, kind="ExternalOutput").ap()
    return io


def build_nc():
    if "nc" in _NC_CACHE:
        return _NC_CACHE["nc"]
    nc = bacc.Bacc("TRN2", target_bir_lowering=False, debug=False,
                   num_devices=N_CORES)
    io = build_io(nc)
    with tile.TileContext(nc) as tc:
        emit(tc, io, H, W)
    nc.compile()
    _NC_CACHE["nc"] = nc
    return nc


def kernel(x, offset_w, offset_b, conv_w, bn_gamma, bn_beta, bn_mean, bn_var):
    x = np.asarray(x, np.float32)
    offset_w = np.asarray(offset_w, np.float32)
    offset_b = np.asarray(offset_b, np.float32)
    conv_w = np.asarray(conv_w, np.float32)
    bn_gamma = np.asarray(bn_gamma, np.float32)
    bn_beta = np.asarray(bn_beta, np.float32)
    bn_mean = np.asarray(bn_mean, np.float32)
    bn_var = np.asarray(bn_var, np.float32)
    B = x.shape[0]
    nc = build_nc()
    shared = prep_shared(offset_w, offset_b, conv_w, bn_gamma, bn_beta,
                         bn_mean, bn_var)
    in_maps = []
    for b in range(B):
        m = dict(shared)
        m.update(prep_x(x[b]))
        in_maps.append(m)
    res = bass_utils.run_bass_kernel_spmd(nc, in_maps,
                                          core_ids=list(range(B)))
    out = np.stack([
        np.asarray(res.results[b]["y"], dtype=np.float32).reshape(C, H, W)
        for b in range(B)])
    return out
